# revision 40
# baseline (speedup 1.0000x reference)
"""MLA (multi-latent attention) Trainium2 kernel.

Sharding: 8 cores. Launch A: token-sharded A-projections (8 x 512 tokens,
2 batches x 4 blocks). Launch B: 2 (batch) x 4 (head-groups of 4 heads);
each core does its 4 heads' B-projections + RoPE + causal attention + a
partial dense contraction; host sums the 4 partials per batch.

v4 design notes (on top of v3):
- Causal slicing: for diagonal key-tiles (m = kt-4qb >= 0) every query
  column j < 128m is fully masked, so the score matmuls, exp, denominator
  accumulate and AV matmuls all operate on the column slice [128m, 512).
  The only remaining mask is the 128x128 triangle on the first valid
  strip, applied for BOTH heads in one N=256 identity-matmul against a
  doubled-triangle tile through a strided 2-region PSUM AP.
- The two heads of a pair share one [128, 2, 512] 2-bank PSUM score tile;
  exp is a single batched ACTIVATE over both heads (the ~293ns fixed
  ACT overhead is paid once, not twice). The AV matmuls lag the exp by
  TWO key-tiles so the PE never waits on the ACT exp pipeline.
- The pair's denominator reduces land on PSUM rows 0-31/32-63 of one
  [64,512] tile so reciprocal + bf16 downcast run once per pair (DVE op
  cost is free-dim-bound: [64,512] costs the same as [1,512]).
- Engine placement: exp + recb broadcast-evac on ACT, denominator adds +
  aoT evacuation + reciprocal on DVE, attention-output normalize multiply
  on GpSimd (SBUF-only op; GpSimd is otherwise idle). Head-pair 0's
  normalizer broadcast/multiply tail is deferred into head-pair 1's kt
  loop to fill PE bubbles.
- All DMA rides the two hardware DGE queues (sync + scalar); the GpSimd
  software ring (and its end-of-launch DRAIN stalls) is gone. One
  consumption-ordered sync stream with the first tiles split small so the
  opening matmul group starts as early as possible; consts + dense
  weights queue behind the critical tiles to avoid HBM contention.
- Launch A: kv projection runs in two 256-token halves with the norm +
  store of half 0 overlapped by half 1's matmuls (short tail); q-rot is
  projected before q-nope in launch B so the RoPE chain hides under the
  Qn matmuls.
"""

import os
import sys

import numpy as np

for _p in ("/opt/trn_rl_repo",):
    if _p not in sys.path:
        sys.path.insert(0, _p)

import ml_dtypes  # noqa: E402

import concourse.bass as bass  # noqa: E402
import concourse.tile as tile  # noqa: E402
from concourse import bacc  # noqa: E402
from concourse import mybir  # noqa: E402
from concourse.bass import ts  # noqa: E402
from concourse.bass_utils import run_bass_kernel_spmd  # noqa: E402

BF16 = mybir.dt.bfloat16
FP32 = mybir.dt.float32

B, S, HID = 2, 2048, 2048
H = 16
NOPE, ROPE, V = 128, 64, 128
QL, KVL = 1536, 512
SCALE = (NOPE + ROPE) ** -0.5
EPS = 1e-6

HPG = 4          # heads per group (per core)
D = NOPE + ROPE  # 192 per-head q/k dim
NT = S // 128    # 16 token tiles of 128
NB = S // 512    # 4 token blocks of 512

NQL = QL // 128   # 12
NKV = KVL // 128  # 4
NHS = HID // 128  # 16

LAST_A = None
LAST_B = None


def _rope_inplace(nc, q, rh, cos_sb, sinT_sb, nb64):
    """In-place RoPE on q [64*nb64, ...]: q = q*cos + rot_half(q)*sinT.

    sinT is sign-baked: rows 0:32 hold -sin, rows 32:64 hold +sin (the
    sin table rows repeat with period 32), which folds rotate_half's
    negation into the table. Partition-shifted reads are only legal for
    single-input ops, so the shift is a copy. rh is scratch shaped like q.
    """
    for blk in range(nb64):
        p0 = 64 * blk
        nc.vector.tensor_copy(rh[p0:p0 + 32], q[p0 + 32:p0 + 64])
        nc.vector.tensor_copy(rh[p0 + 32:p0 + 64], q[p0:p0 + 32])
    nc.vector.tensor_mul(rh[:], rh[:], sinT_sb[:])
    nc.vector.tensor_mul(q[:], q[:], cos_sb[:])
    nc.vector.tensor_add(q[:], q[:], rh[:])


def _emit_a(tc):
    """Launch A: token-sharded A-projections (512 tokens per core)."""
    nc = tc.nc
    TS = 512  # tokens per core

    h_in = nc.dram_tensor("h_t", [128, NHS * TS], BF16, kind="ExternalInput").ap()
    qa_in = nc.dram_tensor("qa_t", [NQL, 128, NHS * 128], BF16,
                           kind="ExternalInput").ap()
    kva_in = nc.dram_tensor("kva_t", [NKV + 1, 128, NHS * 128], BF16,
                            kind="ExternalInput").ap()
    cosA_in = nc.dram_tensor("cosA", [ROPE, TS], BF16, kind="ExternalInput").ap()
    sinTA_in = nc.dram_tensor("sinTA", [ROPE, TS], BF16, kind="ExternalInput").ap()
    qn_out = nc.dram_tensor("qn", [QL, TS], BF16, kind="ExternalOutput").ap()
    ckv_out = nc.dram_tensor("ckv", [KVL + ROPE, TS], BF16, kind="ExternalOutput").ap()

    qn_r = qn_out.rearrange("(j p) t -> p j t", p=128)

    with (
        tc.tile_pool(name="consts", bufs=1) as consts,
        tc.tile_pool(name="ph", bufs=1) as ph,
        tc.tile_pool(name="plat", bufs=1) as plat,
        tc.tile_pool(name="pw", bufs=1) as pw,
        tc.tile_pool(name="pscr", bufs=4) as pscr,
        tc.tile_pool(name="pnorm", bufs=2) as pnorm,
        tc.tile_pool(name="pp_mm", bufs=5, space="PSUM") as pp_mm,
        tc.tile_pool(name="pp_sq", bufs=3, space="PSUM") as pp_sq,
    ):
        # Sync carries the whole bulk stream in first-consumption order
        # (qa tile 0 + h in 4-tile chunks so the first matmul group can
        # start ~5us earlier); scalar only the tiny late-needed tables, so
        # neither queue's transfers contend with the critical path.
        h_sb = ph.tile([128, NHS, TS], BF16)
        qa_sb = pw.tile([128, NQL, NHS * 128], BF16)
        kva_sb = pw.tile([128, NKV + 1, NHS * 128], BF16)
        nc.sync.dma_start(qa_sb[:, 0, 0:8 * 128], qa_in[0, :, 0:8 * 128])
        nc.sync.dma_start(h_sb[:, 0:2, :], h_in[:, 0:2 * TS])
        nc.sync.dma_start(qa_sb[:, 0, 8 * 128:], qa_in[0, :, 8 * 128:])
        nc.sync.dma_start(h_sb[:, 2:4, :], h_in[:, 2 * TS:4 * TS])
        for c in range(1, 4):
            nc.sync.dma_start(h_sb[:, 4 * c:4 * (c + 1), :],
                              h_in[:, 4 * c * TS:4 * (c + 1) * TS])
        for j in range(1, NQL):
            nc.sync.dma_start(qa_sb[:, j, :], qa_in[j])
        for j in range(NKV + 1):
            nc.sync.dma_start(kva_sb[:, j, :], kva_in[j])

        cosA_sb = consts.tile([ROPE, TS], BF16)
        nc.scalar.dma_start(cosA_sb[:], cosA_in)
        sinTA_sb = consts.tile([ROPE, TS], BF16)
        nc.scalar.dma_start(sinTA_sb[:], sinTA_in)

        ones_k_sb = consts.tile([128, 1], BF16)
        nc.vector.memset(ones_k_sb[:], 1.0)
        ones_b_sb = consts.tile([1, 128], BF16)
        nc.vector.memset(ones_b_sb[:], 1.0)
        eps_sb = consts.tile([1, 1], FP32)
        nc.vector.memset(eps_sb[:], EPS)

        qlat = plat.tile([128, NQL, TS], BF16)
        ckv = plat.tile([128, NKV + 1, TS], BF16)

        def proj(w_sb, n_j, dst, sq_ps, do_sq, js=None, pend_sq=None,
                 t0=0, tw=TS):
            """Projection (token cols [t0, t0+tw)) with the RMS
            square-reduce pipelined one group behind the matmuls (the sq
            ones-matmul otherwise bubbles the PE while waiting on the ACT
            square). `js` selects a chunk of the j range; returns the
            pending square for chunked calls."""
            w_r = {j: w_sb[:, j, :].rearrange("p (k c) -> p k c", c=128)
                   for j in (js if js is not None else range(n_j))}
            sq_js = [j for j in range(n_j) if do_sq(j)]
            for j in (js if js is not None else range(n_j)):
                ps = pp_mm.tile([128, tw], FP32, tag="mm")
                # k ascending: k<4 operands (first h chunk) arrive first
                for k in range(NHS):
                    nc.tensor.matmul(
                        ps[:], w_r[j][:, k, :], h_sb[:, k, t0:t0 + tw],
                        start=(k == 0), stop=(k == NHS - 1),
                    )
                nc.scalar.copy(dst[:, j, t0:t0 + tw], ps[:])
                if pend_sq is not None:
                    pj, sq = pend_sq
                    nc.tensor.matmul(sq_ps[:], ones_k_sb[:], sq[:],
                                     start=(pj == sq_js[0]),
                                     stop=(pj == sq_js[-1]))
                    pend_sq = None
                if do_sq(j):
                    sq = pscr.tile([128, tw], BF16, tag="sq")
                    nc.scalar.square(sq[:], ps[:])
                    pend_sq = (j, sq)
            if js is None and pend_sq is not None:
                pj, sq = pend_sq
                nc.tensor.matmul(sq_ps[:], ones_k_sb[:], sq[:],
                                 start=(pj == sq_js[0]), stop=(pj == sq_js[-1]))
                pend_sq = None
            return pend_sq

        def norm(sq_ps, nfeat, tiles, t0=0, tw=TS):
            std = pnorm.tile([1, tw], FP32, tag="std")
            nc.scalar.activation(std[:], sq_ps[:],
                                 mybir.ActivationFunctionType.Sqrt,
                                 bias=eps_sb[:], scale=1.0 / nfeat)
            inv32 = pnorm.tile([1, tw], FP32, tag="inv32")
            nc.vector.reciprocal_approx_fast(inv32[:], std[:])
            # bf16 downcast keeps the broadcast matmul off the 4x-slow
            # fp32 PE path
            inv = pnorm.tile([1, tw], BF16, tag="inv")
            nc.vector.tensor_copy(inv[:], inv32[:])
            psb = pp_mm.tile([128, tw], FP32, tag="mm")
            nc.tensor.matmul(psb[:], ones_b_sb[:], inv[:], start=True, stop=True)
            bc = pnorm.tile([128, tw], BF16, tag="bc")
            nc.scalar.copy(bc[:], psb[:])
            for t in tiles:
                nc.vector.tensor_mul(t[:, t0:t0 + tw], t[:, t0:t0 + tw], bc[:])

        sq_q = pp_sq.tile([1, TS], FP32, tag="sq1", name="sq_q")
        proj(qa_sb, NQL, qlat, sq_q, lambda j: True)

        # kv projection runs in two 256-token halves so the second half's
        # norm + DMA-out tail is short; the first halves' groups also fill
        # the PE while the q-norm chain (sqrt -> recip -> downcast) runs
        # on ACT/DVE
        HW2 = TS // 2
        sq_k0 = pp_sq.tile([1, HW2], FP32, tag="sq1", name="sq_k0")
        sq_k1 = pp_sq.tile([1, HW2], FP32, tag="sq1", name="sq_k1")
        kv_pend = proj(kva_sb, NKV + 1, ckv, sq_k0, lambda j: j < NKV,
                       js=[0, 1], t0=0, tw=HW2)
        norm(sq_q, QL, [qlat[:, j, :] for j in range(NQL)])
        for j in range(NQL):
            nc.sync.dma_start(qn_r[:, j, :], qlat[:, j, :])
        proj(kva_sb, NKV + 1, ckv, sq_k0, lambda j: j < NKV,
             js=[2, 3, 4], pend_sq=kv_pend, t0=0, tw=HW2)

        krot = ckv[0:ROPE, NKV, :]
        rh_k = pscr.tile([ROPE, TS], BF16, tag="rhk")
        # half 1's first projection groups keep the PE fed while half 0's
        # K-RoPE + norm chain runs on ACT/DVE
        kv_pend1 = proj(kva_sb, NKV + 1, ckv, sq_k1, lambda j: j < NKV,
                        js=[0, 1], t0=HW2, tw=HW2)
        _rope_inplace(nc, krot[:, 0:HW2], rh_k[:, 0:HW2],
                      cosA_sb[:, 0:HW2], sinTA_sb[:, 0:HW2], 1)
        nc.scalar.dma_start(ckv_out[KVL:KVL + ROPE, 0:HW2], krot[:, 0:HW2])
        norm(sq_k0, KVL, [ckv[:, j, :] for j in range(NKV)], t0=0, tw=HW2)
        for j in range(NKV):
            eng = nc.sync if j % 2 == 0 else nc.scalar
            eng.dma_start(ckv_out[ts(j, 128), 0:HW2], ckv[:, j, 0:HW2])

        proj(kva_sb, NKV + 1, ckv, sq_k1, lambda j: j < NKV,
             js=[2, 3, 4], pend_sq=kv_pend1, t0=HW2, tw=HW2)
        # norm chain first so it isn't queued behind the RoPE ops on DVE
        norm(sq_k1, KVL, [ckv[:, j, :] for j in range(NKV)], t0=HW2, tw=HW2)
        _rope_inplace(nc, krot[:, HW2:TS], rh_k[:, HW2:TS],
                      cosA_sb[:, HW2:TS], sinTA_sb[:, HW2:TS], 1)
        nc.scalar.dma_start(ckv_out[KVL:KVL + ROPE, HW2:TS], krot[:, HW2:TS])
        for j in range(NKV):
            eng = nc.sync if j % 2 == 0 else nc.scalar
            eng.dma_start(ckv_out[ts(j, 128), HW2:TS], ckv[:, j, HW2:TS])


def _emit_b(tc):
    """Launch B: B-projections + RoPE + attention + partial dense."""
    nc = tc.nc

    qn_in = nc.dram_tensor("qn_t", [NQL, 128, S], BF16, kind="ExternalInput").ap()
    ckv_in = nc.dram_tensor("ckv_t", [NKV, 128, S], BF16, kind="ExternalInput").ap()
    krot_in = nc.dram_tensor("krot", [ROPE, S], BF16, kind="ExternalInput").ap()
    cosD_in = nc.dram_tensor("cosD", [128, S], BF16, kind="ExternalInput").ap()
    sinTD_in = nc.dram_tensor("sinTD", [128, S], BF16, kind="ExternalInput").ap()
    qb_in = nc.dram_tensor("qb_t", [NQL, 128, HPG * D], BF16,
                           kind="ExternalInput").ap()
    kvb_in = nc.dram_tensor("kvb_t", [NKV, 128, HPG * (NOPE + V)], BF16,
                            kind="ExternalInput").ap()
    dw_in = nc.dram_tensor("dw_t", [128, HPG * HID], BF16, kind="ExternalInput").ap()
    tri_in = nc.dram_tensor("tri", [128, 256], BF16, kind="ExternalInput").ap()
    ident_in = nc.dram_tensor("ident", [128, 128], BF16, kind="ExternalInput").ap()
    out = nc.dram_tensor("partial", [S, HID], BF16, kind="ExternalOutput").ap()

    consts = tc.alloc_tile_pool(name="consts", bufs=1)
    plat = tc.alloc_tile_pool(name="lat", bufs=1, side="right")

    cos_sb = consts.tile([128, S], BF16)
    sinT_sb = consts.tile([128, S], BF16)
    tri_sb = consts.tile([128, 256], BF16)
    ident_sb = consts.tile([128, 128], BF16)
    ones_b_sb = consts.tile([64, 128], BF16)   # all-ones; rows 0/32 used
    ones_k_sb = consts.tile([128, 32], BF16)   # all-ones; 32-wide reduce
    nc.vector.memset(ones_k_sb[:], 1.0)
    nc.vector.memset(ones_b_sb[:], 1.0)

    q_latT = plat.tile([128, NQL, S], BF16)
    ckvT = plat.tile([128, NKV, S], BF16)

    pp_mm = tc.alloc_tile_pool(name="pp_mm", bufs=8, space="PSUM")
    pwb = tc.alloc_tile_pool(name="pwb", bufs=1)
    qb_sb = pwb.tile([128, NQL, HPG * D], BF16)
    kvb_sb = pwb.tile([128, NKV, HPG * (NOPE + V)], BF16)
    dw_sb = pwb.tile([128, HPG, HID], BF16)

    # One sync HW-queue stream in first-consumption order, with the very
    # first tile split so the opening Kn matmuls start early. Keeping the
    # consts/dense-w behind the critical tiles avoids HBM contention.
    nc.sync.dma_start(ckvT[:, 0, 0:512], ckv_in[0, :, 0:512])
    nc.sync.dma_start(kvb_sb[:, 0, 0:NOPE], kvb_in[0, :, 0:NOPE])
    nc.sync.dma_start(ckvT[:, 0, 512:1024], ckv_in[0, :, 512:1024])
    nc.sync.dma_start(kvb_sb[:, 0, NOPE:], kvb_in[0, :, NOPE:])
    nc.sync.dma_start(ckvT[:, 0, 1024:S], ckv_in[0, :, 1024:S])
    for j in range(1, NKV):
        nc.sync.dma_start(ckvT[:, j, :], ckv_in[j])
        nc.sync.dma_start(kvb_sb[:, j, :], kvb_in[j])
    for j in range(NQL):
        nc.sync.dma_start(q_latT[:, j, :], qn_in[j])
        nc.sync.dma_start(qb_sb[:, j, :], qb_in[j])

    # ================= Phase 2a: B-projections ==================
    pqkv = tc.alloc_tile_pool(name="pqkv", bufs=1)
    with (
        tc.tile_pool(name="prope", bufs=1) as prope,
    ):
        # attention operands (built here in phase 2a, used in 2b)
        Qn = pqkv.tile([128, HPG, S], BF16)    # q nope, [d, t] per head
        QrP = pqkv.tile([128, 2, S], BF16)     # q rot, head-paired [2*64, t]
        Kn = pqkv.tile([128, HPG, S], BF16)    # k nope per head
        Vsb = pqkv.tile([128, NT, HPG * V], BF16)  # v, token-major
        KrF2 = pqkv.tile([128, S], BF16)  # rot k rows duplicated to both halves
        nc.sync.dma_start(cos_sb[:], cosD_in)
        nc.sync.dma_start(sinT_sb[:], sinTD_in)
        nc.sync.dma_start(KrF2[0:ROPE, :], krot_in)
        nc.sync.dma_start(KrF2[ROPE:2 * ROPE, :], krot_in)
        nc.sync.dma_start(tri_sb[:], tri_in)
        nc.sync.dma_start(ident_sb[:], ident_in)
        nc.sync.dma_start(dw_sb[:], dw_in)

        # K nope first (its operands are smallest and DMA'd first)
        for h in range(HPG):
            pss = [pp_mm.tile([128, 512], FP32, tag="mm",
                              name=f"kn_ps{h}_{tb}") for tb in range(NB)]
            for j in range(NKV):
                for tb in range(NB):
                    nc.tensor.matmul(
                        pss[tb][:],
                        kvb_sb[:, j, h * (NOPE + V):h * (NOPE + V) + NOPE],
                        ckvT[:, j, ts(tb, 512)],
                        start=(j == 0), stop=(j == NKV - 1),
                    )
            for tb in range(NB):
                # alternate evacuation engine: faster PSUM ring turnover
                if tb % 2 == 0:
                    nc.scalar.copy(Kn[:, h, ts(tb, 512)], pss[tb][:])
                else:
                    nc.vector.tensor_copy(Kn[:, h, ts(tb, 512)], pss[tb][:])

        # V (token-major): out[t, v4] = ckv^T-tile.T @ kvb_v
        kvb_hc = [kvb_sb[:, j, :].rearrange("p (h c) -> p h c", c=NOPE + V)
                  for j in range(NKV)]
        for i in range(NT):
            ps = pp_mm.tile([128, 512], FP32, tag="mm")
            for j in range(NKV):
                nc.tensor.matmul(
                    ps[:], ckvT[:, j, ts(i, 128)],
                    kvb_hc[j][:, :, NOPE:],
                    start=(j == 0), stop=(j == NKV - 1),
                )
            if i % 2 == 0:
                nc.scalar.copy(Vsb[:, i, :], ps[:])
            else:
                nc.vector.tensor_copy(Vsb[:, i, :], ps[:])

        # Q rot, head-paired (M=128 matmuls); then RoPE
        for p in range(2):
            pss = [pp_mm.tile([128, 512], FP32, tag="mm",
                              name=f"qr_ps{p}_{tb}") for tb in range(NB)]
            for j in range(NQL):
                for tb in range(NB):
                    nc.tensor.matmul(
                        pss[tb][:],
                        qb_sb[:, j, HPG * NOPE + p * 128:HPG * NOPE + (p + 1) * 128],
                        q_latT[:, j, ts(tb, 512)],
                        start=(j == 0), stop=(j == NQL - 1),
                    )
            for tb in range(NB):
                nc.scalar.copy(QrP[:, p, ts(tb, 512)], pss[tb][:])
            rh = prope.tile([128, S], BF16, tag="rh")
            _rope_inplace(nc, QrP[:, p, :], rh, cos_sb, sinT_sb, 2)

        # Q nope per head (tb innermost: weight-stationary)
        for h in range(HPG):
            pss = [pp_mm.tile([128, 512], FP32, tag="mm",
                              name=f"qn_ps{h}_{tb}") for tb in range(NB)]
            for j in range(NQL):
                for tb in range(NB):
                    nc.tensor.matmul(
                        pss[tb][:], qb_sb[:, j, h * NOPE:(h + 1) * NOPE],
                        q_latT[:, j, ts(tb, 512)],
                        start=(j == 0), stop=(j == NQL - 1),
                    )
            for tb in range(NB):
                nc.scalar.copy(Qn[:, h, ts(tb, 512)], pss[tb][:])

    pp_mm.release()
    plat.release()

    # ================= Phase 2b: attention + dense ==================
    with (
        tc.tile_pool(name="pao", bufs=1) as pao,
        tc.tile_pool(name="pexp", bufs=6) as pexp,
        tc.tile_pool(name="pfin", bufs=6) as pfin,
        tc.tile_pool(name="pacc", bufs=3) as pacc,
        tc.tile_pool(name="pout", bufs=8) as pout,
        tc.tile_pool(name="pp_s", bufs=2, space="PSUM") as pp_s,
        tc.tile_pool(name="pp_o", bufs=1, space="PSUM") as pp_o,
        tc.tile_pool(name="pp_d", bufs=2, space="PSUM") as pp_d,
    ):
        aoT = pao.tile([128, HPG, S], BF16)  # attn out, [v, t] per head

        def norm_tail(rec, h0, h1, qb=None):
            for i, h in ((0, h0), (1, h1)):
                ps_b = pp_d.tile([128, 512], FP32, tag="d",
                                 name=f"psb{h}_{rec.name}")
                nc.tensor.matmul(ps_b[:], ones_b_sb[32 * i:32 * i + 1, :],
                                 rec[32 * i:32 * i + 1, :],
                                 start=True, stop=True)
                recb = pfin.tile([128, 512], BF16, tag="recb")
                nc.vector.tensor_copy(recb[:], ps_b[:])
                ao_sl = aoT[:, h, ts(qb, 512)]
                nc.gpsimd.tensor_mul(ao_sl, ao_sl, recb[:])

        for qb in range(NB):
            nk = 4 * (qb + 1)
            deferred = None
            for hp in range(2):      # head pair: heads (2hp, 2hp+1)
                h0, h1 = 2 * hp, 2 * hp + 1
                ps_o = pp_o.tile([128, 2, 512], FP32, tag="o",
                                 name=f"o{qb}_{hp}")
                acc = pacc.tile([128, 2, 512], BF16, tag="acc",
                                name=f"acc{qb}_{hp}")
                q0 = 512 * qb
                pend = []
                for kt in range(nk):
                    m = kt - 4 * qb
                    lo = 0 if m < 0 else 128 * m
                    # scores for both heads into one 2-bank PSUM tile,
                    # column-sliced to the causally live region
                    pss = pp_s.tile([128, 2, 512], FP32, tag="s",
                                    name=f"s{qb}_{hp}_{kt}")
                    nc.tensor.matmul(pss[:, 0, lo:], Kn[:, h0, ts(kt, 128)],
                                     Qn[:, h0, q0 + lo:q0 + 512],
                                     start=True, stop=False)
                    nc.tensor.matmul(pss[:, 1, lo:], Kn[:, h1, ts(kt, 128)],
                                     Qn[:, h1, q0 + lo:q0 + 512],
                                     start=True, stop=False)
                    # the pair's two K=64 rot matmuls sit in disjoint row
                    # groups (0-63 / 64-127) and co-issue on the PE
                    nc.tensor.matmul(pss[:, 0, lo:], KrF2[0:64, ts(kt, 128)],
                                     QrP[0:64, hp, q0 + lo:q0 + 512],
                                     start=False, stop=(m < 0))
                    nc.tensor.matmul(pss[:, 1, lo:], KrF2[64:128, ts(kt, 128)],
                                     QrP[64:128, hp, q0 + lo:q0 + 512],
                                     start=False, stop=(m < 0))
                    if m >= 0:
                        # additive causal triangle on both heads' first live
                        # strips in one matmul (strided 2-region out AP)
                        nc.tensor.matmul(pss[:, :, lo:lo + 128], ident_sb[:],
                                         tri_sb[:], start=False, stop=True)
                    e = pexp.tile([128, 2, 512], BF16, tag="e")
                    nc.scalar.activation(
                        e[:, :, lo:], pss[:, :, lo:],
                        mybir.ActivationFunctionType.Exp,
                        scale=SCALE,
                    )
                    if kt == 0:
                        nc.vector.tensor_copy(acc[:], e[:])
                    else:
                        nc.vector.tensor_add(acc[:, :, lo:], acc[:, :, lo:],
                                             e[:, :, lo:])
                    if len(pend) == 2:
                        pkt, plo, pe = pend.pop(0)
                        nc.tensor.matmul(
                            ps_o[:, 0, plo:], Vsb[:, pkt, ts(h0, V)],
                            pe[:, 0, plo:],
                            start=(pkt == 0), stop=False,
                        )
                        nc.tensor.matmul(
                            ps_o[:, 1, plo:], Vsb[:, pkt, ts(h1, V)],
                            pe[:, 1, plo:],
                            start=(pkt == 0), stop=False,
                        )
                    pend.append((kt, lo, e))
                    if kt == 1 and deferred is not None:
                        norm_tail(*deferred, qb=qb)
                        deferred = None
                for fi, (pkt, plo, pe) in enumerate(pend):
                    last = (fi == len(pend) - 1)
                    nc.tensor.matmul(ps_o[:, 0, plo:], Vsb[:, pkt, ts(h0, V)],
                                     pe[:, 0, plo:], start=(pkt == 0),
                                     stop=last)
                    nc.tensor.matmul(ps_o[:, 1, plo:], Vsb[:, pkt, ts(h1, V)],
                                     pe[:, 1, plo:], start=(pkt == 0),
                                     stop=last)
                # evacuate both heads' attention outputs UNNORMALIZED in
                # one cast (frees the PSUM slot without waiting on the
                # normalizer), then normalize in place on GpSimd. The two
                # denominator reduces land on PSUM rows 0-31/32-63 of one
                # tile so the reciprocal + downcast run once per pair (DVE
                # op cost is free-dim-bound; [64,512] costs the same as
                # [1,512]).
                ps_n = pp_d.tile([64, 512], FP32, tag="d",
                                 name=f"psn{qb}_{hp}")
                last_pair = (qb == NB - 1 and hp == 1)
                if not last_pair:
                    nc.vector.tensor_copy(aoT[:, h0:h0 + 2, ts(qb, 512)],
                                          ps_o[:])
                nc.tensor.matmul(ps_n[0:32, :], ones_k_sb[:], acc[:, 0, :],
                                 start=True, stop=True)
                nc.tensor.matmul(ps_n[32:64, :], ones_k_sb[:], acc[:, 1, :],
                                 start=True, stop=True)
                rec32 = pfin.tile([64, 512], FP32, tag="rec32")
                nc.vector.reciprocal_approx_fast(rec32[:], ps_n[:])
                rec = pfin.tile([64, 512], BF16, tag="rec")
                nc.vector.tensor_copy(rec[:], rec32[:])
                if last_pair:
                    # no next pair waits on ps_o: put the recip chain ahead
                    # of the evacuation so the bcast isn't the tail
                    nc.vector.tensor_copy(aoT[:, h0:h0 + 2, ts(qb, 512)],
                                          ps_o[:])
                if hp == 0:
                    # defer the broadcast/normalize of head-pair 0 into
                    # head-pair 1's kt loop (fills exp-wait PE bubbles)
                    deferred = (rec, h0, h1)
                else:
                    norm_tail(rec, h0, h1, qb=qb)

            # dense for this q-block's 4 token tiles; nb in pairs so the
            # aoT stationary tile is loaded once per (i, nb-pair, h)
            for i in range(4 * qb, 4 * qb + 4):
                for nbp in range(2):
                    ps_d = [pp_d.tile([128, 512], FP32, tag="d",
                                      name=f"d{i}_{nbp}_{k}") for k in range(2)]
                    for h in range(HPG):
                        for k in range(2):
                            nc.tensor.matmul(
                                ps_d[k][:], aoT[:, h, ts(i, 128)],
                                dw_sb[:, h, ts(2 * nbp + k, 512)],
                                start=(h == 0), stop=(h == HPG - 1),
                            )
                    for k in range(2):
                        o_sb = pout.tile([128, 512], BF16, tag="osb")
                        # alternate evacuation engine to spread the load
                        if (i + k) % 2 == 0:
                            nc.vector.tensor_copy(o_sb[:], ps_d[k][:])
                        else:
                            nc.scalar.copy(o_sb[:], ps_d[k][:])
                        eng = nc.scalar if (i >= 14 and k == 1) else nc.sync
                        eng.dma_start(
                            out[ts(i, 128), ts(2 * nbp + k, 512)], o_sb[:]
                        )

    pqkv.release()
    pwb.release()
    consts.release()


_PROG_A = None
_PROG_B = None


def _build2():
    global _PROG_A, _PROG_B
    if _PROG_A is None:
        nc = bacc.Bacc("TRN2", target_bir_lowering=False, debug=False,
                       enable_asserts=False, num_devices=8)
        with tile.TileContext(nc) as tc:
            _emit_a(tc)
        nc.compile()
        _PROG_A = nc
    if _PROG_B is None:
        nc = bacc.Bacc("TRN2", target_bir_lowering=False, debug=False,
                       enable_asserts=False, num_devices=8)
        with tile.TileContext(nc) as tc:
            _emit_b(tc)
        nc.compile()
        _PROG_B = nc
    return _PROG_A, _PROG_B


def _bf16(x):
    return np.ascontiguousarray(np.asarray(x, np.float32)).astype(ml_dtypes.bfloat16)


def _sign_baked_sin(sin_rows):
    """[ROPE, T] fp32 -> sign-baked: rows 0:32 = -sin, 32:64 = +sin."""
    out = np.array(sin_rows, np.float32)
    out[0:32] = -out[0:32]
    return out


def qb_perm_cols(g):
    """q_b column permutation per head-group: nope h0..h3, then rot pairs."""
    cols = []
    base = g * HPG * D
    for h in range(HPG):
        cols.extend(range(base + h * D, base + h * D + NOPE))
    for h in range(HPG):
        cols.extend(range(base + h * D + NOPE, base + (h + 1) * D))
    return np.array(cols)


def kernel(
    hidden_states, cos, sin, q_a_w, q_a_ln, q_b_w, kv_a_w, kv_a_ln, kv_b_w, dense_w
):
    global LAST_A, LAST_B
    prog_a, prog_b = _build2()

    hidden_states = np.asarray(hidden_states, np.float32)
    cos = np.asarray(cos, np.float32)
    sin = np.asarray(sin, np.float32)
    qa = np.asarray(q_a_w, np.float32)
    kva = np.asarray(kv_a_w, np.float32)
    qb_full = np.asarray(q_b_w, np.float32)
    kvb_full = np.asarray(kv_b_w, np.float32)
    dw_full = np.asarray(dense_w, np.float32)

    ident = np.eye(128, dtype=np.float32).astype(ml_dtypes.bfloat16)
    jj = np.arange(128)[None, :]
    pp = np.arange(128)[:, None]
    tri1 = np.where(jj >= pp, 0.0, -30000.0).astype(np.float32)
    tri = np.concatenate([tri1, tri1], axis=1).astype(ml_dtypes.bfloat16)

    # pretile A weights: [j, p, k*128+c] = w[k*128+p, j*128+c]
    qa_t = _bf16(np.ascontiguousarray(
        qa.reshape(NHS, 128, NQL, 128).transpose(2, 1, 0, 3)
        .reshape(NQL, 128, NHS * 128)))
    kva_pad = np.zeros((HID, (NKV + 1) * 128), np.float32)
    kva_pad[:, :KVL + ROPE] = kva
    kva_t = _bf16(np.ascontiguousarray(
        kva_pad.reshape(NHS, 128, NKV + 1, 128).transpose(2, 1, 0, 3)
        .reshape(NKV + 1, 128, NHS * 128)))

    # ---- launch A: token-sharded A-projections ----
    in_maps_a = []
    for c in range(8):
        b, t4 = divmod(c, 4)
        tok = slice(t4 * 512, (t4 + 1) * 512)
        hs = hidden_states[b][tok, :]  # [512, HID]
        h_t = _bf16(np.ascontiguousarray(
            hs.T.reshape(NHS, 128, 512).transpose(1, 0, 2).reshape(128, NHS * 512)))
        in_maps_a.append(dict(
            h_t=h_t, qa_t=qa_t, kva_t=kva_t,
            cosA=_bf16(cos[b][tok].T), sinTA=_bf16(_sign_baked_sin(sin[b][tok].T)),
        ))
    res_a = run_bass_kernel_spmd(prog_a, in_maps_a, list(range(8)))
    LAST_A = res_a

    # host: assemble full latents per batch, pretiled for B
    qn_t = []
    ckv_t = []
    krot = []
    for b in range(B):
        qnT = np.concatenate([res_a.results[4 * b + t]["qn"] for t in range(4)],
                             axis=1)  # [QL, S] bf16
        ckvT = np.concatenate([res_a.results[4 * b + t]["ckv"] for t in range(4)],
                              axis=1)  # [KVL+ROPE, S]
        qn_t.append(np.ascontiguousarray(qnT.reshape(NQL, 128, S)))
        ckv_t.append(np.ascontiguousarray(ckvT[:KVL].reshape(NKV, 128, S)))
        krot.append(np.ascontiguousarray(ckvT[KVL:]))

    cosD = np.tile(cos.transpose(0, 2, 1), (1, 2, 1))  # [B, 128, S]
    sinD = np.tile(sin.transpose(0, 2, 1), (1, 2, 1))
    sinD[:, 0:32] = -sinD[:, 0:32]
    sinD[:, 64:96] = -sinD[:, 64:96]

    in_maps_b = []
    for c in range(8):
        b, g = divmod(c, 4)
        qb_slice = qb_full[:, qb_perm_cols(g)]  # [QL, 768]
        qb_t = _bf16(np.ascontiguousarray(qb_slice.reshape(NQL, 128, HPG * D)))
        kvb_slice = kvb_full[:, g * HPG * (NOPE + V):(g + 1) * HPG * (NOPE + V)]
        kvb_t = _bf16(np.ascontiguousarray(
            kvb_slice.reshape(NKV, 128, HPG * (NOPE + V))))
        dw_slice = dw_full[g * HPG * V:(g + 1) * HPG * V, :]  # [512, HID]
        dw_t = _bf16(np.ascontiguousarray(
            dw_slice.reshape(HPG, 128, HID).transpose(1, 0, 2)
            .reshape(128, HPG * HID)))
        in_maps_b.append(dict(
            qn_t=qn_t[b], ckv_t=ckv_t[b], krot=krot[b],
            cosD=_bf16(cosD[b]), sinTD=_bf16(sinD[b]),
            qb_t=qb_t, kvb_t=kvb_t, dw_t=dw_t,
            tri=tri, ident=ident,
        ))
    res_b = run_bass_kernel_spmd(prog_b, in_maps_b, list(range(8)))
    LAST_B = res_b

    out = np.zeros((B, S, HID), np.float32)
    for c in range(8):
        out[c // 4] += res_b.results[c]["partial"].astype(np.float32)
    return out


if __name__ == "__main__":
    _build2()
    print("programs built OK")


# revision 41
# speedup vs baseline: 1.0063x; 1.0063x over previous
"""MLA (multi-latent attention) Trainium2 kernel.

Sharding: 8 cores. Launch A: token-sharded A-projections (8 x 512 tokens,
2 batches x 4 blocks). Launch B: 2 (batch) x 4 (head-groups of 4 heads);
each core does its 4 heads' B-projections + RoPE + causal attention + a
partial dense contraction; host sums the 4 partials per batch.

v4 design notes (on top of v3):
- Causal slicing: for diagonal key-tiles (m = kt-4qb >= 0) every query
  column j < 128m is fully masked, so the score matmuls, exp, denominator
  accumulate and AV matmuls all operate on the column slice [128m, 512).
  The only remaining mask is the 128x128 triangle on the first valid
  strip, applied for BOTH heads in one N=256 identity-matmul against a
  doubled-triangle tile through a strided 2-region PSUM AP.
- The two heads of a pair share one [128, 2, 512] 2-bank PSUM score tile;
  exp is a single batched ACTIVATE over both heads (the ~293ns fixed
  ACT overhead is paid once, not twice). The AV matmuls lag the exp by
  TWO key-tiles so the PE never waits on the ACT exp pipeline.
- The pair's denominator reduces land on PSUM rows 0-31/32-63 of one
  [64,512] tile so reciprocal + bf16 downcast run once per pair (DVE op
  cost is free-dim-bound: [64,512] costs the same as [1,512]).
- Engine placement: exp + recb broadcast-evac on ACT, denominator adds +
  aoT evacuation + reciprocal on DVE, attention-output normalize multiply
  on GpSimd (SBUF-only op; GpSimd is otherwise idle). Head-pair 0's
  normalizer broadcast/multiply tail is deferred into head-pair 1's kt
  loop to fill PE bubbles.
- All DMA rides the two hardware DGE queues (sync + scalar); the GpSimd
  software ring (and its end-of-launch DRAIN stalls) is gone. One
  consumption-ordered sync stream with the first tiles split small so the
  opening matmul group starts as early as possible; consts + dense
  weights queue behind the critical tiles to avoid HBM contention.
- Launch A: kv projection runs in two 256-token halves with the norm +
  store of half 0 overlapped by half 1's matmuls (short tail); q-rot is
  projected before q-nope in launch B so the RoPE chain hides under the
  Qn matmuls.
"""

import os
import sys

import numpy as np

for _p in ("/opt/trn_rl_repo",):
    if _p not in sys.path:
        sys.path.insert(0, _p)

import ml_dtypes  # noqa: E402

import concourse.bass as bass  # noqa: E402
import concourse.tile as tile  # noqa: E402
from concourse import bacc  # noqa: E402
from concourse import mybir  # noqa: E402
from concourse.bass import ts  # noqa: E402
from concourse.bass_utils import run_bass_kernel_spmd  # noqa: E402

BF16 = mybir.dt.bfloat16
FP32 = mybir.dt.float32

B, S, HID = 2, 2048, 2048
H = 16
NOPE, ROPE, V = 128, 64, 128
QL, KVL = 1536, 512
SCALE = (NOPE + ROPE) ** -0.5
EPS = 1e-6

HPG = 4          # heads per group (per core)
D = NOPE + ROPE  # 192 per-head q/k dim
NT = S // 128    # 16 token tiles of 128
NB = S // 512    # 4 token blocks of 512

NQL = QL // 128   # 12
NKV = KVL // 128  # 4
NHS = HID // 128  # 16

LAST_A = None
LAST_B = None


def _rope_inplace(nc, q, rh, cos_sb, sinT_sb, nb64):
    """In-place RoPE on q [64*nb64, ...]: q = q*cos + rot_half(q)*sinT.

    sinT is sign-baked: rows 0:32 hold -sin, rows 32:64 hold +sin (the
    sin table rows repeat with period 32), which folds rotate_half's
    negation into the table. Partition-shifted reads are only legal for
    single-input ops, so the shift is a copy. rh is scratch shaped like q.
    """
    for blk in range(nb64):
        p0 = 64 * blk
        nc.vector.tensor_copy(rh[p0:p0 + 32], q[p0 + 32:p0 + 64])
        nc.vector.tensor_copy(rh[p0 + 32:p0 + 64], q[p0:p0 + 32])
    nc.vector.tensor_mul(rh[:], rh[:], sinT_sb[:])
    nc.vector.tensor_mul(q[:], q[:], cos_sb[:])
    nc.vector.tensor_add(q[:], q[:], rh[:])


def _emit_a(tc):
    """Launch A: token-sharded A-projections (512 tokens per core)."""
    nc = tc.nc
    TS = 512  # tokens per core

    h_in = nc.dram_tensor("h_t", [128, NHS * TS], BF16, kind="ExternalInput").ap()
    qa_in = nc.dram_tensor("qa_t", [NQL, 128, NHS * 128], BF16,
                           kind="ExternalInput").ap()
    kva_in = nc.dram_tensor("kva_t", [NKV + 1, 128, NHS * 128], BF16,
                            kind="ExternalInput").ap()
    cosA_in = nc.dram_tensor("cosA", [ROPE, TS], BF16, kind="ExternalInput").ap()
    sinTA_in = nc.dram_tensor("sinTA", [ROPE, TS], BF16, kind="ExternalInput").ap()
    qn_out = nc.dram_tensor("qn", [QL, TS], BF16, kind="ExternalOutput").ap()
    ckv_out = nc.dram_tensor("ckv", [KVL + ROPE, TS], BF16, kind="ExternalOutput").ap()

    qn_r = qn_out.rearrange("(j p) t -> p j t", p=128)

    with (
        tc.tile_pool(name="consts", bufs=1) as consts,
        tc.tile_pool(name="ph", bufs=1) as ph,
        tc.tile_pool(name="plat", bufs=1) as plat,
        tc.tile_pool(name="pw", bufs=1) as pw,
        tc.tile_pool(name="pscr", bufs=4) as pscr,
        tc.tile_pool(name="pnorm", bufs=2) as pnorm,
        tc.tile_pool(name="pp_mm", bufs=5, space="PSUM") as pp_mm,
        tc.tile_pool(name="pp_sq", bufs=3, space="PSUM") as pp_sq,
    ):
        # Sync carries the whole bulk stream in first-consumption order
        # (qa tile 0 + h in 4-tile chunks so the first matmul group can
        # start ~5us earlier); scalar only the tiny late-needed tables, so
        # neither queue's transfers contend with the critical path.
        h_sb = ph.tile([128, NHS, TS], BF16)
        qa_sb = pw.tile([128, NQL, NHS * 128], BF16)
        kva_sb = pw.tile([128, NKV + 1, NHS * 128], BF16)
        nc.sync.dma_start(qa_sb[:, 0, 0:8 * 128], qa_in[0, :, 0:8 * 128])
        nc.sync.dma_start(h_sb[:, 0:2, :], h_in[:, 0:2 * TS])
        nc.sync.dma_start(qa_sb[:, 0, 8 * 128:], qa_in[0, :, 8 * 128:])
        nc.sync.dma_start(h_sb[:, 2:4, :], h_in[:, 2 * TS:4 * TS])
        for c in range(1, 4):
            nc.sync.dma_start(h_sb[:, 4 * c:4 * (c + 1), :],
                              h_in[:, 4 * c * TS:4 * (c + 1) * TS])
        for j in range(1, NQL):
            nc.sync.dma_start(qa_sb[:, j, :], qa_in[j])
        for j in range(NKV + 1):
            nc.sync.dma_start(kva_sb[:, j, :], kva_in[j])

        cosA_sb = consts.tile([ROPE, TS], BF16)
        nc.scalar.dma_start(cosA_sb[:], cosA_in)
        sinTA_sb = consts.tile([ROPE, TS], BF16)
        nc.scalar.dma_start(sinTA_sb[:], sinTA_in)

        ones_k_sb = consts.tile([128, 1], BF16)
        nc.vector.memset(ones_k_sb[:], 1.0)
        ones_b_sb = consts.tile([1, 128], BF16)
        nc.vector.memset(ones_b_sb[:], 1.0)
        eps_sb = consts.tile([1, 1], FP32)
        nc.vector.memset(eps_sb[:], EPS)

        qlat = plat.tile([128, NQL, TS], BF16)
        ckv = plat.tile([128, NKV + 1, TS], BF16)

        def proj(w_sb, n_j, dst, sq_ps, do_sq, js=None, pend_sq=None,
                 t0=0, tw=TS):
            """Projection (token cols [t0, t0+tw)) with the RMS
            square-reduce pipelined one group behind the matmuls (the sq
            ones-matmul otherwise bubbles the PE while waiting on the ACT
            square). `js` selects a chunk of the j range; returns the
            pending square for chunked calls."""
            w_r = {j: w_sb[:, j, :].rearrange("p (k c) -> p k c", c=128)
                   for j in (js if js is not None else range(n_j))}
            sq_js = [j for j in range(n_j) if do_sq(j)]
            for j in (js if js is not None else range(n_j)):
                ps = pp_mm.tile([128, tw], FP32, tag="mm")
                # k ascending: k<4 operands (first h chunk) arrive first
                for k in range(NHS):
                    nc.tensor.matmul(
                        ps[:], w_r[j][:, k, :], h_sb[:, k, t0:t0 + tw],
                        start=(k == 0), stop=(k == NHS - 1),
                    )
                nc.scalar.copy(dst[:, j, t0:t0 + tw], ps[:])
                if pend_sq is not None:
                    pj, sq = pend_sq
                    nc.tensor.matmul(sq_ps[:], ones_k_sb[:], sq[:],
                                     start=(pj == sq_js[0]),
                                     stop=(pj == sq_js[-1]))
                    pend_sq = None
                if do_sq(j):
                    sq = pscr.tile([128, tw], BF16, tag="sq")
                    nc.scalar.square(sq[:], ps[:])
                    pend_sq = (j, sq)
            if js is None and pend_sq is not None:
                pj, sq = pend_sq
                nc.tensor.matmul(sq_ps[:], ones_k_sb[:], sq[:],
                                 start=(pj == sq_js[0]), stop=(pj == sq_js[-1]))
                pend_sq = None
            return pend_sq

        def norm(sq_ps, nfeat, tiles, t0=0, tw=TS):
            std = pnorm.tile([1, tw], FP32, tag="std")
            nc.scalar.activation(std[:], sq_ps[:],
                                 mybir.ActivationFunctionType.Sqrt,
                                 bias=eps_sb[:], scale=1.0 / nfeat)
            inv32 = pnorm.tile([1, tw], FP32, tag="inv32")
            nc.vector.reciprocal_approx_fast(inv32[:], std[:])
            # bf16 downcast keeps the broadcast matmul off the 4x-slow
            # fp32 PE path
            inv = pnorm.tile([1, tw], BF16, tag="inv")
            nc.vector.tensor_copy(inv[:], inv32[:])
            psb = pp_mm.tile([128, tw], FP32, tag="mm")
            nc.tensor.matmul(psb[:], ones_b_sb[:], inv[:], start=True, stop=True)
            bc = pnorm.tile([128, tw], BF16, tag="bc")
            nc.scalar.copy(bc[:], psb[:])
            for t in tiles:
                nc.vector.tensor_mul(t[:, t0:t0 + tw], t[:, t0:t0 + tw], bc[:])

        sq_q = pp_sq.tile([1, TS], FP32, tag="sq1", name="sq_q")
        proj(qa_sb, NQL, qlat, sq_q, lambda j: True)

        # kv projection runs in two 256-token halves so the second half's
        # norm + DMA-out tail is short; the first halves' groups also fill
        # the PE while the q-norm chain (sqrt -> recip -> downcast) runs
        # on ACT/DVE
        HW2 = TS // 2
        sq_k0 = pp_sq.tile([1, HW2], FP32, tag="sq1", name="sq_k0")
        sq_k1 = pp_sq.tile([1, HW2], FP32, tag="sq1", name="sq_k1")
        kv_pend = proj(kva_sb, NKV + 1, ckv, sq_k0, lambda j: j < NKV,
                       js=[0, 1], t0=0, tw=HW2)
        norm(sq_q, QL, [qlat[:, j, :] for j in range(NQL)])
        for j in range(NQL):
            nc.sync.dma_start(qn_r[:, j, :], qlat[:, j, :])
        proj(kva_sb, NKV + 1, ckv, sq_k0, lambda j: j < NKV,
             js=[2, 3, 4], pend_sq=kv_pend, t0=0, tw=HW2)

        krot = ckv[0:ROPE, NKV, :]
        rh_k = pscr.tile([ROPE, TS], BF16, tag="rhk")
        # half 1's first projection groups keep the PE fed while half 0's
        # K-RoPE + norm chain runs on ACT/DVE
        kv_pend1 = proj(kva_sb, NKV + 1, ckv, sq_k1, lambda j: j < NKV,
                        js=[0, 1], t0=HW2, tw=HW2)
        _rope_inplace(nc, krot[:, 0:HW2], rh_k[:, 0:HW2],
                      cosA_sb[:, 0:HW2], sinTA_sb[:, 0:HW2], 1)
        nc.scalar.dma_start(ckv_out[KVL:KVL + ROPE, 0:HW2], krot[:, 0:HW2])
        norm(sq_k0, KVL, [ckv[:, j, :] for j in range(NKV)], t0=0, tw=HW2)
        for j in range(NKV):
            eng = nc.sync if j % 2 == 0 else nc.scalar
            eng.dma_start(ckv_out[ts(j, 128), 0:HW2], ckv[:, j, 0:HW2])

        proj(kva_sb, NKV + 1, ckv, sq_k1, lambda j: j < NKV,
             js=[2, 3, 4], pend_sq=kv_pend1, t0=HW2, tw=HW2)
        # norm chain first so it isn't queued behind the RoPE ops on DVE
        norm(sq_k1, KVL, [ckv[:, j, :] for j in range(NKV)], t0=HW2, tw=HW2)
        _rope_inplace(nc, krot[:, HW2:TS], rh_k[:, HW2:TS],
                      cosA_sb[:, HW2:TS], sinTA_sb[:, HW2:TS], 1)
        nc.scalar.dma_start(ckv_out[KVL:KVL + ROPE, HW2:TS], krot[:, HW2:TS])
        for j in range(NKV):
            eng = nc.sync if j % 2 == 0 else nc.scalar
            eng.dma_start(ckv_out[ts(j, 128), HW2:TS], ckv[:, j, HW2:TS])


def _emit_b(tc):
    """Launch B: B-projections + RoPE + attention + partial dense."""
    nc = tc.nc

    qn_in = nc.dram_tensor("qn_t", [NQL, 128, S], BF16, kind="ExternalInput").ap()
    ckv_in = nc.dram_tensor("ckv_t", [NKV, 128, S], BF16, kind="ExternalInput").ap()
    krot_in = nc.dram_tensor("krot", [ROPE, S], BF16, kind="ExternalInput").ap()
    cosD_in = nc.dram_tensor("cosD", [128, S], BF16, kind="ExternalInput").ap()
    sinTD_in = nc.dram_tensor("sinTD", [128, S], BF16, kind="ExternalInput").ap()
    qb_in = nc.dram_tensor("qb_t", [NQL, 128, HPG * D], BF16,
                           kind="ExternalInput").ap()
    kvb_in = nc.dram_tensor("kvb_t", [NKV, 128, HPG * (NOPE + V)], BF16,
                            kind="ExternalInput").ap()
    dw_in = nc.dram_tensor("dw_t", [128, HPG * HID], BF16, kind="ExternalInput").ap()
    tri_in = nc.dram_tensor("tri", [128, 256], BF16, kind="ExternalInput").ap()
    ident_in = nc.dram_tensor("ident", [128, 128], BF16, kind="ExternalInput").ap()
    out = nc.dram_tensor("partial", [S, HID], BF16, kind="ExternalOutput").ap()

    consts = tc.alloc_tile_pool(name="consts", bufs=1)
    plat = tc.alloc_tile_pool(name="lat", bufs=1, side="right")

    cos_sb = consts.tile([128, S], BF16)
    sinT_sb = consts.tile([128, S], BF16)
    tri_sb = consts.tile([128, 256], BF16)
    ident_sb = consts.tile([128, 128], BF16)
    ones_b_sb = consts.tile([64, 128], BF16)   # all-ones; rows 0/32 used
    ones_k_sb = consts.tile([128, 32], BF16)   # all-ones; 32-wide reduce
    nc.vector.memset(ones_k_sb[:], 1.0)
    nc.vector.memset(ones_b_sb[:], 1.0)

    q_latT = plat.tile([128, NQL, S], BF16)
    ckvT = plat.tile([128, NKV, S], BF16)

    pp_mm = tc.alloc_tile_pool(name="pp_mm", bufs=8, space="PSUM")
    pwb = tc.alloc_tile_pool(name="pwb", bufs=1)
    qb_sb = pwb.tile([128, NQL, HPG * D], BF16)
    kvb_sb = pwb.tile([128, NKV, HPG * (NOPE + V)], BF16)
    dw_sb = pwb.tile([128, HPG, HID], BF16)

    # One sync HW-queue stream in first-consumption order, with the very
    # first tile split so the opening Kn matmuls start early. Keeping the
    # consts/dense-w behind the critical tiles avoids HBM contention.
    nc.sync.dma_start(ckvT[:, 0, 0:512], ckv_in[0, :, 0:512])
    nc.sync.dma_start(kvb_sb[:, 0, 0:NOPE], kvb_in[0, :, 0:NOPE])
    nc.sync.dma_start(ckvT[:, 0, 512:1024], ckv_in[0, :, 512:1024])
    nc.sync.dma_start(kvb_sb[:, 0, NOPE:], kvb_in[0, :, NOPE:])
    nc.sync.dma_start(ckvT[:, 0, 1024:S], ckv_in[0, :, 1024:S])
    for j in range(1, NKV):
        nc.sync.dma_start(ckvT[:, j, :], ckv_in[j])
        nc.sync.dma_start(kvb_sb[:, j, :], kvb_in[j])
    for j in range(NQL):
        nc.sync.dma_start(q_latT[:, j, :], qn_in[j])
        nc.sync.dma_start(qb_sb[:, j, :], qb_in[j])

    # ================= Phase 2a: B-projections ==================
    pqkv = tc.alloc_tile_pool(name="pqkv", bufs=1)
    with (
        tc.tile_pool(name="prope", bufs=1) as prope,
    ):
        # attention operands (built here in phase 2a, used in 2b)
        Qn = pqkv.tile([128, HPG, S], BF16)    # q nope, [d, t] per head
        QrP = pqkv.tile([128, 2, S], BF16)     # q rot, head-paired [2*64, t]
        Kn = pqkv.tile([128, HPG, S], BF16)    # k nope per head
        Vsb = pqkv.tile([128, NT, HPG * V], BF16)  # v, token-major
        KrF2 = pqkv.tile([128, S], BF16)  # rot k rows duplicated to both halves
        nc.sync.dma_start(cos_sb[:], cosD_in)
        nc.sync.dma_start(sinT_sb[:], sinTD_in)
        nc.sync.dma_start(KrF2[0:ROPE, :], krot_in)
        nc.sync.dma_start(KrF2[ROPE:2 * ROPE, :], krot_in)
        nc.sync.dma_start(tri_sb[:], tri_in)
        nc.sync.dma_start(ident_sb[:], ident_in)
        nc.sync.dma_start(dw_sb[:], dw_in)

        # K nope first (its operands are smallest and DMA'd first)
        for h in range(HPG):
            pss = [pp_mm.tile([128, 512], FP32, tag="mm",
                              name=f"kn_ps{h}_{tb}") for tb in range(NB)]
            for j in range(NKV):
                for tb in range(NB):
                    nc.tensor.matmul(
                        pss[tb][:],
                        kvb_sb[:, j, h * (NOPE + V):h * (NOPE + V) + NOPE],
                        ckvT[:, j, ts(tb, 512)],
                        start=(j == 0), stop=(j == NKV - 1),
                    )
            for tb in range(NB):
                # alternate evacuation engine: faster PSUM ring turnover
                if tb % 2 == 0:
                    nc.scalar.copy(Kn[:, h, ts(tb, 512)], pss[tb][:])
                else:
                    nc.vector.tensor_copy(Kn[:, h, ts(tb, 512)], pss[tb][:])

        # V (token-major): out[t, v4] = ckv^T-tile.T @ kvb_v
        kvb_hc = [kvb_sb[:, j, :].rearrange("p (h c) -> p h c", c=NOPE + V)
                  for j in range(NKV)]
        for i in range(NT):
            ps = pp_mm.tile([128, 512], FP32, tag="mm")
            for j in range(NKV):
                nc.tensor.matmul(
                    ps[:], ckvT[:, j, ts(i, 128)],
                    kvb_hc[j][:, :, NOPE:],
                    start=(j == 0), stop=(j == NKV - 1),
                )
            if i % 2 == 0:
                nc.scalar.copy(Vsb[:, i, :], ps[:])
            else:
                nc.vector.tensor_copy(Vsb[:, i, :], ps[:])

        # Q rot, head-paired (M=128 matmuls); then RoPE
        for p in range(2):
            pss = [pp_mm.tile([128, 512], FP32, tag="mm",
                              name=f"qr_ps{p}_{tb}") for tb in range(NB)]
            for j in range(NQL):
                for tb in range(NB):
                    nc.tensor.matmul(
                        pss[tb][:],
                        qb_sb[:, j, HPG * NOPE + p * 128:HPG * NOPE + (p + 1) * 128],
                        q_latT[:, j, ts(tb, 512)],
                        start=(j == 0), stop=(j == NQL - 1),
                    )
            for tb in range(NB):
                nc.scalar.copy(QrP[:, p, ts(tb, 512)], pss[tb][:])
            rh = prope.tile([128, S], BF16, tag="rh")
            _rope_inplace(nc, QrP[:, p, :], rh, cos_sb, sinT_sb, 2)

        # Q nope per head (tb innermost: weight-stationary)
        for h in range(HPG):
            pss = [pp_mm.tile([128, 512], FP32, tag="mm",
                              name=f"qn_ps{h}_{tb}") for tb in range(NB)]
            for j in range(NQL):
                for tb in range(NB):
                    nc.tensor.matmul(
                        pss[tb][:], qb_sb[:, j, h * NOPE:(h + 1) * NOPE],
                        q_latT[:, j, ts(tb, 512)],
                        start=(j == 0), stop=(j == NQL - 1),
                    )
            for tb in range(NB):
                if tb % 2 == 0:
                    nc.scalar.copy(Qn[:, h, ts(tb, 512)], pss[tb][:])
                else:
                    nc.vector.tensor_copy(Qn[:, h, ts(tb, 512)], pss[tb][:])

    pp_mm.release()
    plat.release()

    # ================= Phase 2b: attention + dense ==================
    with (
        tc.tile_pool(name="pao", bufs=1) as pao,
        tc.tile_pool(name="pexp", bufs=6) as pexp,
        tc.tile_pool(name="pfin", bufs=6) as pfin,
        tc.tile_pool(name="pacc", bufs=3) as pacc,
        tc.tile_pool(name="pout", bufs=8) as pout,
        tc.tile_pool(name="pp_s", bufs=2, space="PSUM") as pp_s,
        tc.tile_pool(name="pp_o", bufs=1, space="PSUM") as pp_o,
        tc.tile_pool(name="pp_d", bufs=2, space="PSUM") as pp_d,
    ):
        aoT = pao.tile([128, HPG, S], BF16)  # attn out, [v, t] per head

        def norm_tail(rec, h0, h1, qb=None):
            for i, h in ((0, h0), (1, h1)):
                ps_b = pp_d.tile([128, 512], FP32, tag="d",
                                 name=f"psb{h}_{rec.name}")
                nc.tensor.matmul(ps_b[:], ones_b_sb[32 * i:32 * i + 1, :],
                                 rec[32 * i:32 * i + 1, :],
                                 start=True, stop=True)
                recb = pfin.tile([128, 512], BF16, tag="recb")
                nc.scalar.copy(recb[:], ps_b[:])
                ao_sl = aoT[:, h, ts(qb, 512)]
                nc.gpsimd.tensor_mul(ao_sl, ao_sl, recb[:])

        for qb in range(NB):
            nk = 4 * (qb + 1)
            deferred = None
            for hp in range(2):      # head pair: heads (2hp, 2hp+1)
                h0, h1 = 2 * hp, 2 * hp + 1
                ps_o = pp_o.tile([128, 2, 512], FP32, tag="o",
                                 name=f"o{qb}_{hp}")
                acc = pacc.tile([128, 2, 512], BF16, tag="acc",
                                name=f"acc{qb}_{hp}")
                q0 = 512 * qb
                pend = []
                for kt in range(nk):
                    m = kt - 4 * qb
                    lo = 0 if m < 0 else 128 * m
                    # scores for both heads into one 2-bank PSUM tile,
                    # column-sliced to the causally live region
                    pss = pp_s.tile([128, 2, 512], FP32, tag="s",
                                    name=f"s{qb}_{hp}_{kt}")
                    nc.tensor.matmul(pss[:, 0, lo:], Kn[:, h0, ts(kt, 128)],
                                     Qn[:, h0, q0 + lo:q0 + 512],
                                     start=True, stop=False)
                    nc.tensor.matmul(pss[:, 1, lo:], Kn[:, h1, ts(kt, 128)],
                                     Qn[:, h1, q0 + lo:q0 + 512],
                                     start=True, stop=False)
                    # the pair's two K=64 rot matmuls sit in disjoint row
                    # groups (0-63 / 64-127) and co-issue on the PE
                    nc.tensor.matmul(pss[:, 0, lo:], KrF2[0:64, ts(kt, 128)],
                                     QrP[0:64, hp, q0 + lo:q0 + 512],
                                     start=False, stop=(m < 0))
                    nc.tensor.matmul(pss[:, 1, lo:], KrF2[64:128, ts(kt, 128)],
                                     QrP[64:128, hp, q0 + lo:q0 + 512],
                                     start=False, stop=(m < 0))
                    if m >= 0:
                        # additive causal triangle on both heads' first live
                        # strips in one matmul (strided 2-region out AP)
                        nc.tensor.matmul(pss[:, :, lo:lo + 128], ident_sb[:],
                                         tri_sb[:], start=False, stop=True)
                    e = pexp.tile([128, 2, 512], BF16, tag="e")
                    nc.scalar.activation(
                        e[:, :, lo:], pss[:, :, lo:],
                        mybir.ActivationFunctionType.Exp,
                        scale=SCALE,
                    )
                    if kt == 0:
                        nc.vector.tensor_copy(acc[:], e[:])
                    else:
                        nc.vector.tensor_add(acc[:, :, lo:], acc[:, :, lo:],
                                             e[:, :, lo:])
                    if len(pend) == 2:
                        pkt, plo, pe = pend.pop(0)
                        nc.tensor.matmul(
                            ps_o[:, 0, plo:], Vsb[:, pkt, ts(h0, V)],
                            pe[:, 0, plo:],
                            start=(pkt == 0), stop=False,
                        )
                        nc.tensor.matmul(
                            ps_o[:, 1, plo:], Vsb[:, pkt, ts(h1, V)],
                            pe[:, 1, plo:],
                            start=(pkt == 0), stop=False,
                        )
                    pend.append((kt, lo, e))
                    if kt == 1 and deferred is not None:
                        norm_tail(*deferred, qb=qb)
                        deferred = None
                for fi, (pkt, plo, pe) in enumerate(pend):
                    last = (fi == len(pend) - 1)
                    nc.tensor.matmul(ps_o[:, 0, plo:], Vsb[:, pkt, ts(h0, V)],
                                     pe[:, 0, plo:], start=(pkt == 0),
                                     stop=last)
                    nc.tensor.matmul(ps_o[:, 1, plo:], Vsb[:, pkt, ts(h1, V)],
                                     pe[:, 1, plo:], start=(pkt == 0),
                                     stop=last)
                # evacuate both heads' attention outputs UNNORMALIZED in
                # one cast (frees the PSUM slot without waiting on the
                # normalizer), then normalize in place on GpSimd. The two
                # denominator reduces land on PSUM rows 0-31/32-63 of one
                # tile so the reciprocal + downcast run once per pair (DVE
                # op cost is free-dim-bound; [64,512] costs the same as
                # [1,512]).
                ps_n = pp_d.tile([64, 512], FP32, tag="d",
                                 name=f"psn{qb}_{hp}")
                last_pair = (qb == NB - 1 and hp == 1)
                if not last_pair:
                    nc.vector.tensor_copy(aoT[:, h0:h0 + 2, ts(qb, 512)],
                                          ps_o[:])
                nc.tensor.matmul(ps_n[0:32, :], ones_k_sb[:], acc[:, 0, :],
                                 start=True, stop=True)
                nc.tensor.matmul(ps_n[32:64, :], ones_k_sb[:], acc[:, 1, :],
                                 start=True, stop=True)
                rec32 = pfin.tile([64, 512], FP32, tag="rec32")
                nc.vector.reciprocal_approx_fast(rec32[:], ps_n[:])
                rec = pfin.tile([64, 512], BF16, tag="rec")
                nc.vector.tensor_copy(rec[:], rec32[:])
                if last_pair:
                    # no next pair waits on ps_o: put the recip chain ahead
                    # of the evacuation so the bcast isn't the tail
                    nc.vector.tensor_copy(aoT[:, h0:h0 + 2, ts(qb, 512)],
                                          ps_o[:])
                if hp == 0:
                    # defer the broadcast/normalize of head-pair 0 into
                    # head-pair 1's kt loop (fills exp-wait PE bubbles)
                    deferred = (rec, h0, h1)
                else:
                    norm_tail(rec, h0, h1, qb=qb)

            # dense for this q-block's 4 token tiles; nb in pairs so the
            # aoT stationary tile is loaded once per (i, nb-pair, h)
            for i in range(4 * qb, 4 * qb + 4):
                for nbp in range(2):
                    ps_d = [pp_d.tile([128, 512], FP32, tag="d",
                                      name=f"d{i}_{nbp}_{k}") for k in range(2)]
                    for h in range(HPG):
                        for k in range(2):
                            nc.tensor.matmul(
                                ps_d[k][:], aoT[:, h, ts(i, 128)],
                                dw_sb[:, h, ts(2 * nbp + k, 512)],
                                start=(h == 0), stop=(h == HPG - 1),
                            )
                    for k in range(2):
                        o_sb = pout.tile([128, 512], BF16, tag="osb")
                        # alternate evacuation engine to spread the load
                        if (i + k) % 2 == 0:
                            nc.vector.tensor_copy(o_sb[:], ps_d[k][:])
                        else:
                            nc.scalar.copy(o_sb[:], ps_d[k][:])
                        eng = nc.scalar if (i >= 14 and k == 1) else nc.sync
                        eng.dma_start(
                            out[ts(i, 128), ts(2 * nbp + k, 512)], o_sb[:]
                        )

    pqkv.release()
    pwb.release()
    consts.release()


_PROG_A = None
_PROG_B = None


def _build2():
    global _PROG_A, _PROG_B
    if _PROG_A is None:
        nc = bacc.Bacc("TRN2", target_bir_lowering=False, debug=False,
                       enable_asserts=False, num_devices=8)
        with tile.TileContext(nc) as tc:
            _emit_a(tc)
        nc.compile()
        _PROG_A = nc
    if _PROG_B is None:
        nc = bacc.Bacc("TRN2", target_bir_lowering=False, debug=False,
                       enable_asserts=False, num_devices=8)
        with tile.TileContext(nc) as tc:
            _emit_b(tc)
        nc.compile()
        _PROG_B = nc
    return _PROG_A, _PROG_B


def _bf16(x):
    return np.ascontiguousarray(np.asarray(x, np.float32)).astype(ml_dtypes.bfloat16)


def _sign_baked_sin(sin_rows):
    """[ROPE, T] fp32 -> sign-baked: rows 0:32 = -sin, 32:64 = +sin."""
    out = np.array(sin_rows, np.float32)
    out[0:32] = -out[0:32]
    return out


def qb_perm_cols(g):
    """q_b column permutation per head-group: nope h0..h3, then rot pairs."""
    cols = []
    base = g * HPG * D
    for h in range(HPG):
        cols.extend(range(base + h * D, base + h * D + NOPE))
    for h in range(HPG):
        cols.extend(range(base + h * D + NOPE, base + (h + 1) * D))
    return np.array(cols)


def kernel(
    hidden_states, cos, sin, q_a_w, q_a_ln, q_b_w, kv_a_w, kv_a_ln, kv_b_w, dense_w
):
    global LAST_A, LAST_B
    prog_a, prog_b = _build2()

    hidden_states = np.asarray(hidden_states, np.float32)
    cos = np.asarray(cos, np.float32)
    sin = np.asarray(sin, np.float32)
    qa = np.asarray(q_a_w, np.float32)
    kva = np.asarray(kv_a_w, np.float32)
    qb_full = np.asarray(q_b_w, np.float32)
    kvb_full = np.asarray(kv_b_w, np.float32)
    dw_full = np.asarray(dense_w, np.float32)

    ident = np.eye(128, dtype=np.float32).astype(ml_dtypes.bfloat16)
    jj = np.arange(128)[None, :]
    pp = np.arange(128)[:, None]
    tri1 = np.where(jj >= pp, 0.0, -30000.0).astype(np.float32)
    tri = np.concatenate([tri1, tri1], axis=1).astype(ml_dtypes.bfloat16)

    # pretile A weights: [j, p, k*128+c] = w[k*128+p, j*128+c]
    qa_t = _bf16(np.ascontiguousarray(
        qa.reshape(NHS, 128, NQL, 128).transpose(2, 1, 0, 3)
        .reshape(NQL, 128, NHS * 128)))
    kva_pad = np.zeros((HID, (NKV + 1) * 128), np.float32)
    kva_pad[:, :KVL + ROPE] = kva
    kva_t = _bf16(np.ascontiguousarray(
        kva_pad.reshape(NHS, 128, NKV + 1, 128).transpose(2, 1, 0, 3)
        .reshape(NKV + 1, 128, NHS * 128)))

    # ---- launch A: token-sharded A-projections ----
    in_maps_a = []
    for c in range(8):
        b, t4 = divmod(c, 4)
        tok = slice(t4 * 512, (t4 + 1) * 512)
        hs = hidden_states[b][tok, :]  # [512, HID]
        h_t = _bf16(np.ascontiguousarray(
            hs.T.reshape(NHS, 128, 512).transpose(1, 0, 2).reshape(128, NHS * 512)))
        in_maps_a.append(dict(
            h_t=h_t, qa_t=qa_t, kva_t=kva_t,
            cosA=_bf16(cos[b][tok].T), sinTA=_bf16(_sign_baked_sin(sin[b][tok].T)),
        ))
    res_a = run_bass_kernel_spmd(prog_a, in_maps_a, list(range(8)))
    LAST_A = res_a

    # host: assemble full latents per batch, pretiled for B
    qn_t = []
    ckv_t = []
    krot = []
    for b in range(B):
        qnT = np.concatenate([res_a.results[4 * b + t]["qn"] for t in range(4)],
                             axis=1)  # [QL, S] bf16
        ckvT = np.concatenate([res_a.results[4 * b + t]["ckv"] for t in range(4)],
                              axis=1)  # [KVL+ROPE, S]
        qn_t.append(np.ascontiguousarray(qnT.reshape(NQL, 128, S)))
        ckv_t.append(np.ascontiguousarray(ckvT[:KVL].reshape(NKV, 128, S)))
        krot.append(np.ascontiguousarray(ckvT[KVL:]))

    cosD = np.tile(cos.transpose(0, 2, 1), (1, 2, 1))  # [B, 128, S]
    sinD = np.tile(sin.transpose(0, 2, 1), (1, 2, 1))
    sinD[:, 0:32] = -sinD[:, 0:32]
    sinD[:, 64:96] = -sinD[:, 64:96]

    in_maps_b = []
    for c in range(8):
        b, g = divmod(c, 4)
        qb_slice = qb_full[:, qb_perm_cols(g)]  # [QL, 768]
        qb_t = _bf16(np.ascontiguousarray(qb_slice.reshape(NQL, 128, HPG * D)))
        kvb_slice = kvb_full[:, g * HPG * (NOPE + V):(g + 1) * HPG * (NOPE + V)]
        kvb_t = _bf16(np.ascontiguousarray(
            kvb_slice.reshape(NKV, 128, HPG * (NOPE + V))))
        dw_slice = dw_full[g * HPG * V:(g + 1) * HPG * V, :]  # [512, HID]
        dw_t = _bf16(np.ascontiguousarray(
            dw_slice.reshape(HPG, 128, HID).transpose(1, 0, 2)
            .reshape(128, HPG * HID)))
        in_maps_b.append(dict(
            qn_t=qn_t[b], ckv_t=ckv_t[b], krot=krot[b],
            cosD=_bf16(cosD[b]), sinTD=_bf16(sinD[b]),
            qb_t=qb_t, kvb_t=kvb_t, dw_t=dw_t,
            tri=tri, ident=ident,
        ))
    res_b = run_bass_kernel_spmd(prog_b, in_maps_b, list(range(8)))
    LAST_B = res_b

    out = np.zeros((B, S, HID), np.float32)
    for c in range(8):
        out[c // 4] += res_b.results[c]["partial"].astype(np.float32)
    return out


if __name__ == "__main__":
    _build2()
    print("programs built OK")
